# revision 64
# baseline (speedup 1.0000x reference)
"""MHA kernel builder for TRN2 (per-core SPMD program) + host prep.

Problem: out = X + MHA(RMSNorm(X)) where Q=K=V=(RMSNorm(X)@Wq.T+b), rope,
causal softmax, Wo projection. Sharding: batch(2) x head-groups(4) over 8
cores; each core computes a partial of out[b] (its 4 heads through Wo);
host sums partials + bias + residual.

Device-side structure (all matmuls fp16 operands, fp32 PSUM):
- RMS scale folded on host: Xn = X * rsqrt(mean(X^2)+eps); rms_w folded
  into Wq. Device sees pre-normalized XT.
- Phase A: Q projection, two 512-col chunks in flight across all 8 PSUM
  banks so the PE streams with the X DMA.
- Phase B: V = Q^T via PE transposes; rope on DVE.
- Phase C: attention qc-outer / head-inner with Wo interleaved one chunk
  behind, causal-trimmed score/denominator/AV matmuls, additive-free tri
  masking via Pool-engine multiply, reciprocals on ACT.
"""
import math
import itertools
import numpy as np
from contextlib import ExitStack

import concourse.bass as bass
import concourse.mybir as mybir
import concourse.tile as tile

F32 = mybir.dt.float32
F32R = mybir.dt.float32r
F16 = mybir.dt.float16
BF16 = mybir.dt.bfloat16

EPS = float(np.finfo(np.float32).eps)
ROPE_BASE = 10000.0
AF = mybir.ActivationFunctionType

_ctr = itertools.count()


def legalize_sync_waits(nc, max_waits=1):
    """This walrus accepts at most one sync-wait per instruction; hoist
    excess waits onto same-engine NOPs inserted just before."""
    n_fixed = 0
    for f in nc.m.functions:
        for bb in f.blocks:
            insts = bb.instructions
            out = []
            dirty = False
            for inst in insts:
                si = getattr(inst, "sync_info", None)
                if si is not None and si.on_wait and len(si.on_wait) > max_waits:
                    waits = list(si.on_wait)
                    for w in waits[:-max_waits]:
                        nop = mybir.InstNoOp(
                            name=f"I-syncfix-{next(_ctr)}", engine=inst.engine
                        )
                        nop.sync_info = mybir.SyncInfo(on_wait=[w], on_update=[])
                        nc.register_instruction(nop, overwrite=True)
                        out.append(nop)
                    inst.sync_info = mybir.SyncInfo(
                        on_wait=waits[-max_waits:], on_update=list(si.on_update or [])
                    )
                    dirty = True
                    n_fixed += 1
                out.append(inst)
            if dirty:
                bb.instructions = out
    return n_fixed


def build_core(S=2048, D=2048, NHL=4, DK=128, SHIFT=10.0):
    """Emit the per-core program. Returns nc. All cores run this same NEFF
    with different input data."""
    assert S % 512 == 0 and D % 128 == 0 and DK == 128
    SK = S // 512     # 512-wide seq chunks
    KT = D // 128     # contraction tiles for projections
    ST = S // 128     # 128-wide seq tiles
    ML = NHL * DK     # local model width (q columns this core owns)

    nc = bass.Bass("TRN2", num_devices=8)
    dXT = nc.dram_tensor("XT", [D, S], F16, kind="ExternalInput")
    dWQT = nc.dram_tensor("WQT", [D, ML], F16, kind="ExternalInput")
    dWOT = nc.dram_tensor("WOT", [ML, D], F16, kind="ExternalInput")
    dQB = nc.dram_tensor("QB", [128, NHL], F32, kind="ExternalInput")
    dCOSA = nc.dram_tensor("COSA", [DK, S], F16, kind="ExternalInput")
    dSINA = nc.dram_tensor("SINA", [DK, S], F16, kind="ExternalInput")
    dTRI = nc.dram_tensor("TRI", [128, 128], F32, kind="ExternalInput")
    dONEC = nc.dram_tensor("ONEC", [128, 1], F32, kind="ExternalInput")
    dONER = nc.dram_tensor("ONER", [1, 128], F32, kind="ExternalInput")
    dOUT = nc.dram_tensor("OUTP", [S, D], F16, kind="ExternalOutput")

    with tile.TileContext(nc) as tc, ExitStack() as ctx:
        pp = ctx.enter_context(tc.tile_pool(name="pp", bufs=1))

        # ---- constants (always live) ------------------------------------
        cosa = pp.tile([DK, S], F16, name="cosa")
        sina = pp.tile([DK, S], F16, name="sina")
        tri = pp.tile([128, 128], BF16, name="tri")
        onecb = pp.tile([128, 1], BF16, name="onecb")
        onerb = pp.tile([1, 128], BF16, name="onerb")
        shift_t = pp.tile([128, 1], F32, name="shift_t")
        qb = pp.tile([128, NHL], F32, name="qb")
        nc.gpsimd.dma_start(out=cosa, in_=dCOSA[:, :])
        nc.gpsimd.dma_start(out=sina, in_=dSINA[:, :])
        nc.gpsimd.dma_start(out=tri, in_=dTRI[:, :])
        nc.gpsimd.dma_start(out=onecb, in_=dONEC[:, :])
        nc.gpsimd.dma_start(out=onerb, in_=dONER[:, :])
        nc.vector.memset(shift_t, -SHIFT)
        nc.gpsimd.dma_start(out=qb, in_=dQB[:, :])

        pmid = ctx.enter_context(tc.tile_pool(name="pmid", bufs=1))
        qth = [pmid.tile([DK, S], F16, name=f"qth{h}") for h in range(NHL)]
        qr = [pmid.tile([DK, S], F16, name=f"qr{h}") for h in range(NHL)]

        pvv = ctx.enter_context(tc.tile_pool(name="pvv", bufs=1))
        vv = [pvv.tile([128, ST, DK], F16, name=f"vv{h}")
              for h in range(NHL)]
        wo = pvv.tile([128, NHL, D], F16, name="wo")

        def rope_half(h, half):
            """rope + V-transpose for columns [half*1024, half*1024+1024)"""
            sl = slice(half * 1024, half * 1024 + 1024)
            hw = DK // 2
            sh = pvv.tile([DK, 1024], F16, name=f"sh{h}_{half}", tag="ropesh",
                          bufs=2)
            nc.vector.tensor_copy(sh[0:hw, :], qth[h][hw:DK, sl])
            nc.vector.tensor_copy(sh[hw:DK, :], qth[h][0:hw, sl])
            m1 = pvv.tile([DK, 1024], F16, name=f"m1_{h}_{half}",
                          tag="ropem1", bufs=2)
            nc.vector.tensor_mul(m1, qth[h][:, sl], cosa[:, sl])
            nc.vector.tensor_mul(sh, sh, sina[:, sl])
            nc.vector.tensor_add(qr[h][:, sl], m1, sh)
            # V^T via DMA XBAR transpose: vv[p, st, dk] = qth[dk, st*128+p];
            # AV runs mixed fp16(vv) x bf16(probs), so no convert needed
            nc.sync.dma_start_transpose(
                out=vv[h][:, half * (ST // 2):(half + 1) * (ST // 2), :],
                in_=qth[h][:, sl])

        # ---- Phase A: load X/W, Q projection ----------------------------
        with tc.tile_pool(name="pxw", bufs=1) as pxw, \
             tc.tile_pool(name="psQ", bufs=1, space="PSUM") as psQ:
            xt = [pxw.tile([128, S], F16, name=f"xt{k}") for k in range(KT)]
            wq = pxw.tile([128, KT, ML], F16, name="wq")
            # load X column-halves in phase order so the PE starts early;
            # half 1 rides the scalar ring so the sync ring frees up for
            # the V transposes
            for k in range(KT):
                if k < 3:
                    # quarter-granularity for the first tiles so the very
                    # first matmuls start as soon as 256 KB has landed
                    nc.sync.dma_start(out=xt[k][:, 0:512],
                                      in_=dXT[k * 128:(k + 1) * 128, 0:512])
                    nc.sync.dma_start(out=xt[k][:, 512:1024],
                                      in_=dXT[k * 128:(k + 1) * 128, 512:1024])
                else:
                    nc.sync.dma_start(out=xt[k][:, 0:1024],
                                      in_=dXT[k * 128:(k + 1) * 128, 0:1024])
                nc.scalar.dma_start(out=wq[:, k, :],
                                    in_=dWQT[k * 128:(k + 1) * 128, :])
            for k in range(KT):
                nc.scalar.dma_start(out=xt[k][:, 1024:2048],
                                    in_=dXT[k * 128:(k + 1) * 128, 1024:2048])
            # wo lives in the pre-A pool (no SBUF alias with xt/wq), so
            # these loads run early without blocking the scalar ring
            for h in range(NHL):
                nc.scalar.dma_start(out=wo[:, h, :],
                                    in_=dWOT[h * 128:(h + 1) * 128, :])

            for half in range(2):
                cs = (2 * half, 2 * half + 1)
                qps = {}
                for c in cs:
                    for mt in range(NHL):
                        qps[(c, mt)] = psQ.tile([128, 512], F32,
                                                name=f"qps{c}_{mt}",
                                                tag="q", bufs=8)
                # k-runs into the same PSUM bank (216 ns/matmul chain rate;
                # bank switches cost ~43 ns extra). Half 0 starts k-outer
                # (one matmul per group per k) to track the X DMA stream,
                # then lengthens runs; half 1 has X resident and chains 16.
                kranges = ([(k, k + 1) for k in range(10)]
                           + [(10, 13), (13, 16)]) if half == 0 \
                    else [(0, 16)]

                def runs(c, mt, k0, k1):
                    for k in range(k0, k1):
                        nc.tensor.matmul(
                            qps[(c, mt)],
                            wq[:, k, mt * 128:(mt + 1) * 128],
                            xt[k][:, c * 512:(c + 1) * 512],
                            start=(k == 0), stop=(k == KT - 1))

                def drain(c, mt):
                    sl = slice(c * 512, (c + 1) * 512)
                    # drain PSUM + bias; split across DVE and ACT
                    if mt % 2 == 0:
                        nc.vector.tensor_scalar_add(
                            qth[mt][:, sl], qps[(c, mt)], qb[:, mt:mt + 1])
                    else:
                        nc.scalar.add(qth[mt][:, sl], qps[(c, mt)],
                                      qb[:, mt:mt + 1])

                for k0, k1 in kranges[:-1]:
                    for c in cs:
                        for mt in range(NHL):
                            runs(c, mt, k0, k1)
                # last k-range: finish and drain one c at a time so banks
                # free earlier for the next half / attention PSUM pools
                k0, k1 = kranges[-1]
                for c in cs:
                    for mt in range(NHL):
                        runs(c, mt, k0, k1)
                    for mt in range(NHL):
                        drain(c, mt)
                # rope/transpose for this column half overlaps the next
                # half's matmuls on the PE
                for h in range(NHL):
                    rope_half(h, half)

        # ---- Phase C pool -----------------------------------------------
        with tc.tile_pool(name="pback", bufs=1) as pback:
            ot = [pback.tile([DK, S], F16, name=f"ot{h}") for h in range(NHL)]

            # ---- Phase C: attention (qc-outer) + interleaved Wo ---------
            psS = ctx.enter_context(
                tc.tile_pool(name="psS", bufs=1, space="PSUM"))
            psO = ctx.enter_context(
                tc.tile_pool(name="psO", bufs=1, space="PSUM"))
            psW = ctx.enter_context(
                tc.tile_pool(name="psW", bufs=1, space="PSUM"))

            NC = D // 512
            ats_map = {}

            def emit_scores(h, qc):
                npt = qc * 4 + 4
                ats = []
                # score tiles in [128, 2, 512] pairs (2 PSUM banks) so the
                # exp covers 1024 cols per ACT instruction where possible
                for j in range(npt // 2):
                    sp2 = psS.tile([128, 2, 512], F32, name=f"sp{h}_{qc}_{j}",
                                   tag="sps", bufs=2)
                    at2 = pback.tile([128, 2, 512], BF16,
                                     name=f"at{h}_{qc}_{j}",
                                     tag=f"atp{j}", bufs=4)
                    los = []
                    for jj in range(2):
                        pt = 2 * j + jj
                        off = pt * 128 - qc * 512
                        lo = max(0, off)
                        los.append((lo, off))
                        nc.tensor.matmul(
                            sp2[:, jj, lo:512],
                            qr[h][:, pt * 128:(pt + 1) * 128],
                            qr[h][:, qc * 512 + lo:(qc + 1) * 512],
                            start=True, stop=True)
                    # one exp per pair over [lo_min:512] of both halves; the
                    # extra cols of the later-diagonal half are unwritten
                    # PSUM garbage that is never read downstream
                    lo_min = min(los[0][0], los[1][0])
                    nc.scalar.activation(at2[:, :, lo_min:512],
                                         sp2[:, :, lo_min:512],
                                         AF.Exp, bias=shift_t)
                    for jj in range(2):
                        lo, off = los[jj]
                        if off >= 0:
                            # zero p > q inside the diagonal 128-col block
                            nc.gpsimd.tensor_mul(at2[:, jj, lo:lo + 128],
                                                 at2[:, jj, lo:lo + 128], tri)
                        ats.append((at2, jj, lo, 2 * j + jj))
                ats_map[(h, qc)] = ats

            def emit_dnav(h, qc):
                npt = qc * 4 + 4
                ats = ats_map.pop((h, qc))
                # denominator over p (PE ones-reduce, causal-trimmed);
                # dn rides the sps slot rotation to stay within 8 banks
                dn = psS.tile([1, 512], F32, name=f"dn{h}_{qc}", tag="sps",
                              bufs=2)
                for i, (at2, jj, lo, pt) in enumerate(ats):
                    nc.tensor.matmul(dn[:, lo:512], onecb, at2[:, jj, lo:512],
                                     start=(i == 0), stop=(i == npt - 1))
                lnd = pback.tile([1, 512], F32, name=f"lnd{h}_{qc}",
                                 tag="lnd", bufs=2)
                nc.scalar.activation(lnd, dn, AF.Ln)
                rdr = pback.tile([1, 512], BF16, name=f"rdr{h}_{qc}",
                                 tag="rdr", bufs=2)
                nc.scalar.activation(rdr, lnd, AF.Exp, scale=-1.0)
                bc = psS.tile([128, 512], F32, name=f"bc{h}_{qc}", tag="sps",
                              bufs=2)
                nc.tensor.matmul(bc, onerb, rdr, start=True, stop=True)
                rdb = pback.tile([128, 512], F32, name=f"rdb{h}_{qc}",
                                 tag="rdb", bufs=2)
                nc.vector.tensor_copy(rdb, bc)

                # AV (causal-trimmed) + normalize
                o_ps = psO.tile([128, 512], F32, name=f"ops{h}_{qc}",
                                tag="ops", bufs=2)
                for i, (at2, jj, lo, pt) in enumerate(ats):
                    nc.tensor.matmul(o_ps[:, lo:512], vv[h][:, pt, :],
                                     at2[:, jj, lo:512],
                                     start=(i == 0), stop=(i == npt - 1))
                sl = slice(qc * 512, (qc + 1) * 512)
                nc.vector.tensor_mul(ot[h][:, sl], o_ps, rdb)

            def emit_wo(st):
                osb = pback.tile([128, D], F16, name=f"osb{st}", tag="osb",
                                 bufs=2)
                for ncc in range(NC):
                    wo_ps = psW.tile([128, 512], F32, name=f"wops{st}_{ncc}",
                                     tag="wops", bufs=2)
                    for hh in range(NHL):
                        nc.tensor.matmul(
                            wo_ps,
                            ot[hh][:, st * 128:(st + 1) * 128],
                            wo[:, hh, ncc * 512:(ncc + 1) * 512],
                            start=(hh == 0), stop=(hh == NHL - 1))
                    nsl = slice(ncc * 512, (ncc + 1) * 512)
                    if ncc % 2 == 0:
                        nc.vector.tensor_copy(osb[:, nsl], wo_ps)
                    else:
                        nc.scalar.copy(osb[:, nsl], wo_ps)
                    if ncc % 2 == 1:
                        hsl = slice((ncc - 1) * 512, (ncc + 1) * 512)
                        nc.sync.dma_start(
                            out=dOUT[st * 128:(st + 1) * 128, hsl],
                            in_=osb[:, hsl])

            # pipeline: scores emitted one (h, qc) unit ahead of their
            # denominator/AV so the PE has work during the exp/mask chain
            order = [(h, qc) for qc in range(SK) for h in range(NHL)]
            emit_scores(*order[0])
            emit_scores(*order[1])
            emit_scores(*order[2])
            for i, (h, qc) in enumerate(order):
                if i + 3 < len(order):
                    emit_scores(*order[i + 3])
                emit_dnav(h, qc)
                if qc > 0:
                    emit_wo((qc - 1) * 4 + h)
            for st in range((SK - 1) * 4, ST):
                emit_wo(st)

    return nc


# ======================= host-side preparation ===========================

def host_prep(X, Wq_w, Wq_b, Wo_w, Wo_b, rms_w, n_cores=8, NHL=4):
    """Build per-core input maps. X: (B,S,D) fp32."""
    B, S, D = X.shape
    DK = 128
    c = DK ** -0.25
    inv = 1.0 / (ROPE_BASE ** (np.arange(0, DK, 2, dtype=np.float64) / DK))
    ang = np.arange(S, dtype=np.float64)[:, None] * inv[None, :]
    cos = np.concatenate([np.cos(ang), np.cos(ang)], -1)     # (S, DK)
    sin = np.concatenate([np.sin(ang), np.sin(ang)], -1)
    COSA = (cos.T * c).astype(np.float16)                    # (DK, S)
    SINT = (sin.T * c).astype(np.float32)
    SINA = np.concatenate([-SINT[:DK // 2], SINT[DK // 2:]], 0).astype(np.float16)
    TRI = np.triu(np.ones((128, 128), np.float32))                # p <= q
    ONEC = np.ones((128, 1), np.float32)
    ONER = np.ones((1, 128), np.float32)

    # fold rms weight into Wq; fold the rms scale into X (commutes with the
    # projection since it is a per-token scalar)
    Wq_eff = (Wq_w * rms_w[None, :]).astype(np.float32)       # (D, D)
    ms = np.mean(X.astype(np.float32) ** 2, axis=-1, keepdims=True)
    Xn = (X / np.sqrt(ms + EPS)).astype(np.float32)           # (B, S, D)

    in_maps = []
    groups = n_cores // B                                     # head-groups per batch
    ML = NHL * DK
    for core in range(n_cores):
        b = core // groups
        hg = core % groups
        msl = slice(hg * ML, (hg + 1) * ML)
        XT = np.ascontiguousarray(Xn[b].T).astype(np.float16)         # (D, S)
        WQT = np.ascontiguousarray(Wq_eff[msl, :].T).astype(np.float16)   # (D, ML)
        WOT = np.ascontiguousarray(Wo_w[:, msl].T).astype(np.float16)     # (ML, D)
        QB = np.ascontiguousarray(
            Wq_b[msl].reshape(NHL, 128).T).astype(np.float32)             # (128, NHL)
        in_maps.append({
            "XT": XT, "WQT": WQT, "WOT": WOT, "QB": QB,
            "COSA": COSA, "SINA": SINA, "TRI": TRI,
            "ONEC": ONEC, "ONER": ONER,
        })
    return in_maps


def host_reduce(X, Wo_b, results, n_cores=8):
    B, S, D = X.shape
    groups = n_cores // B
    out = np.empty((B, S, D), np.float32)
    for b in range(B):
        acc = X[b].astype(np.float32).copy()
        for hg in range(groups):
            acc += results[b * groups + hg]["OUTP"].astype(np.float32)
        acc += Wo_b[None, :]
        out[b] = acc
    return out


# ======================= public entry point ==============================

_CACHE = {}


def _get_nc():
    if "nc" not in _CACHE:
        nc = build_core(S=2048, D=2048, NHL=4, DK=128, SHIFT=10.0)
        legalize_sync_waits(nc, max_waits=1)
        _CACHE["nc"] = nc
    return _CACHE["nc"]


def kernel(X, Wq_w, Wq_b, Wo_w, Wo_b, rms_w):
    """Full-input MHA block: returns X + MHA(RMSNorm(X)) as np.float32.

    Shards batch(2) x head-groups(4) across 8 NeuronCores; each core
    produces a partial output (its 4 heads through Wo); the host sums the
    four partials per batch and adds bias + residual.
    """
    from concourse.bass_utils import run_bass_kernel_spmd

    X = np.asarray(X, np.float32)
    Wq_w = np.asarray(Wq_w, np.float32)
    Wq_b = np.asarray(Wq_b, np.float32)
    Wo_w = np.asarray(Wo_w, np.float32)
    Wo_b = np.asarray(Wo_b, np.float32)
    rms_w = np.asarray(rms_w, np.float32)

    nc = _get_nc()
    in_maps = host_prep(X, Wq_w, Wq_b, Wo_w, Wo_b, rms_w)
    res = run_bass_kernel_spmd(nc, in_maps, core_ids=list(range(8)))
    return host_reduce(X, Wo_b, res.results)


# revision 65
# speedup vs baseline: 1.2287x; 1.2287x over previous
"""MHA kernel builder for TRN2 (per-core SPMD program) + host prep.

Problem: out = X + MHA(RMSNorm(X)) where Q=K=V=(RMSNorm(X)@Wq.T+b), rope,
causal softmax, Wo projection. Sharding: batch(2) x head-groups(4) over 8
cores; each core computes a partial of out[b] (its 4 heads through Wo);
host sums partials + bias + residual.

Device-side structure (all matmuls fp16 operands, fp32 PSUM):
- RMS scale folded on host: Xn = X * rsqrt(mean(X^2)+eps); rms_w folded
  into Wq. Device sees pre-normalized XT.
- Phase A: Q projection, two 512-col chunks in flight across all 8 PSUM
  banks so the PE streams with the X DMA.
- Phase B: V = Q^T via PE transposes; rope on DVE.
- Phase C: attention qc-outer / head-inner with Wo interleaved one chunk
  behind, causal-trimmed score/denominator/AV matmuls, additive-free tri
  masking via Pool-engine multiply, reciprocals on ACT.
"""
import math
import itertools
import numpy as np
from contextlib import ExitStack

import concourse.bass as bass
import concourse.mybir as mybir
import concourse.tile as tile

F32 = mybir.dt.float32
F32R = mybir.dt.float32r
F16 = mybir.dt.float16
BF16 = mybir.dt.bfloat16

EPS = float(np.finfo(np.float32).eps)
ROPE_BASE = 10000.0
AF = mybir.ActivationFunctionType

_ctr = itertools.count()


def legalize_sync_waits(nc, max_waits=1):
    """This walrus accepts at most one sync-wait per instruction; hoist
    excess waits onto same-engine NOPs inserted just before."""
    n_fixed = 0
    for f in nc.m.functions:
        for bb in f.blocks:
            insts = bb.instructions
            out = []
            dirty = False
            for inst in insts:
                si = getattr(inst, "sync_info", None)
                if si is not None and si.on_wait and len(si.on_wait) > max_waits:
                    waits = list(si.on_wait)
                    for w in waits[:-max_waits]:
                        nop = mybir.InstNoOp(
                            name=f"I-syncfix-{next(_ctr)}", engine=inst.engine
                        )
                        nop.sync_info = mybir.SyncInfo(on_wait=[w], on_update=[])
                        nc.register_instruction(nop, overwrite=True)
                        out.append(nop)
                    inst.sync_info = mybir.SyncInfo(
                        on_wait=waits[-max_waits:], on_update=list(si.on_update or [])
                    )
                    dirty = True
                    n_fixed += 1
                out.append(inst)
            if dirty:
                bb.instructions = out
    return n_fixed


def build_core(S=2048, D=2048, NHL=4, DK=128, SHIFT=10.0):
    """Emit the per-core program. Returns nc. All cores run this same NEFF
    with different input data."""
    assert S % 512 == 0 and D % 128 == 0 and DK == 128
    SK = S // 512     # 512-wide seq chunks
    KT = D // 128     # contraction tiles for projections
    ST = S // 128     # 128-wide seq tiles
    ML = NHL * DK     # local model width (q columns this core owns)

    nc = bass.Bass("TRN2", num_devices=8)
    dXT = nc.dram_tensor("XT", [D, S], F16, kind="ExternalInput")
    dWQT = nc.dram_tensor("WQT", [D, ML], F16, kind="ExternalInput")
    dWOT = nc.dram_tensor("WOT", [ML, D], F16, kind="ExternalInput")
    dQB = nc.dram_tensor("QB", [128, NHL], F32, kind="ExternalInput")
    dCOSA = nc.dram_tensor("COSA", [DK, S], F16, kind="ExternalInput")
    dSINA = nc.dram_tensor("SINA", [DK, S], F16, kind="ExternalInput")
    dTRI = nc.dram_tensor("TRI", [128, 128], F32, kind="ExternalInput")
    dONEC = nc.dram_tensor("ONEC", [128, 1], F32, kind="ExternalInput")
    dONER = nc.dram_tensor("ONER", [1, 128], F32, kind="ExternalInput")
    dOUT = nc.dram_tensor("OUTP", [S, D], F16, kind="ExternalOutput")

    with tile.TileContext(nc) as tc, ExitStack() as ctx:
        pp = ctx.enter_context(tc.tile_pool(name="pp", bufs=1))

        # ---- constants (always live) ------------------------------------
        cosa = pp.tile([DK, S], F16, name="cosa")
        sina = pp.tile([DK, S], F16, name="sina")
        tri = pp.tile([128, 128], BF16, name="tri")
        onecb = pp.tile([128, 1], BF16, name="onecb")
        onerb = pp.tile([1, 128], BF16, name="onerb")
        shift_t = pp.tile([128, 1], F32, name="shift_t")
        qb = pp.tile([128, NHL], F32, name="qb")
        nc.gpsimd.dma_start(out=cosa, in_=dCOSA[:, :])
        nc.gpsimd.dma_start(out=sina, in_=dSINA[:, :])
        nc.gpsimd.dma_start(out=tri, in_=dTRI[:, :])
        nc.gpsimd.dma_start(out=onecb, in_=dONEC[:, :])
        nc.gpsimd.dma_start(out=onerb, in_=dONER[:, :])
        nc.vector.memset(shift_t, -SHIFT)
        nc.gpsimd.dma_start(out=qb, in_=dQB[:, :])

        pmid = ctx.enter_context(tc.tile_pool(name="pmid", bufs=1))
        qth = [pmid.tile([DK, S], F16, name=f"qth{h}") for h in range(NHL)]
        qr = [pmid.tile([DK, S], F16, name=f"qr{h}") for h in range(NHL)]

        pvv = ctx.enter_context(tc.tile_pool(name="pvv", bufs=1))
        vv = [pvv.tile([128, ST, DK], F16, name=f"vv{h}")
              for h in range(NHL)]
        wo = pvv.tile([128, NHL, D], F16, name="wo")

        def rope_half(h, half):
            """rope + V-transpose for columns [half*1024, half*1024+1024)"""
            sl = slice(half * 1024, half * 1024 + 1024)
            hw = DK // 2
            sh = pvv.tile([DK, 1024], F16, name=f"sh{h}_{half}", tag="ropesh",
                          bufs=2)
            nc.vector.tensor_copy(sh[0:hw, :], qth[h][hw:DK, sl])
            nc.vector.tensor_copy(sh[hw:DK, :], qth[h][0:hw, sl])
            m1 = pvv.tile([DK, 1024], F16, name=f"m1_{h}_{half}",
                          tag="ropem1", bufs=2)
            nc.vector.tensor_mul(m1, qth[h][:, sl], cosa[:, sl])
            nc.vector.tensor_mul(sh, sh, sina[:, sl])
            nc.vector.tensor_add(qr[h][:, sl], m1, sh)
            # V^T via DMA XBAR transpose: vv[p, st, dk] = qth[dk, st*128+p];
            # AV runs mixed fp16(vv) x bf16(probs), so no convert needed
            nc.sync.dma_start_transpose(
                out=vv[h][:, half * (ST // 2):(half + 1) * (ST // 2), :],
                in_=qth[h][:, sl])

        # ---- Phase A: load X/W, Q projection ----------------------------
        with tc.tile_pool(name="pxw", bufs=1) as pxw, \
             tc.tile_pool(name="psQ", bufs=1, space="PSUM") as psQ:
            xt = [pxw.tile([128, S], F16, name=f"xt{k}") for k in range(KT)]
            wq = pxw.tile([128, KT, ML], F16, name="wq")
            # load X column-halves in phase order so the PE starts early;
            # half 1 rides the scalar ring so the sync ring frees up for
            # the V transposes
            for k in range(KT):
                if k < 3:
                    # quarter-granularity for the first tiles so the very
                    # first matmuls start as soon as 256 KB has landed
                    nc.sync.dma_start(out=xt[k][:, 0:512],
                                      in_=dXT[k * 128:(k + 1) * 128, 0:512])
                    nc.sync.dma_start(out=xt[k][:, 512:1024],
                                      in_=dXT[k * 128:(k + 1) * 128, 512:1024])
                else:
                    nc.sync.dma_start(out=xt[k][:, 0:1024],
                                      in_=dXT[k * 128:(k + 1) * 128, 0:1024])
                nc.scalar.dma_start(out=wq[:, k, :],
                                    in_=dWQT[k * 128:(k + 1) * 128, :])
            for k in range(KT):
                nc.scalar.dma_start(out=xt[k][:, 1024:2048],
                                    in_=dXT[k * 128:(k + 1) * 128, 1024:2048])
            # wo lives in the pre-A pool (no SBUF alias with xt/wq), so
            # these loads run early without blocking the scalar ring
            for h in range(NHL):
                nc.scalar.dma_start(out=wo[:, h, :],
                                    in_=dWOT[h * 128:(h + 1) * 128, :])

            for half in range(2):
                cs = (2 * half, 2 * half + 1)
                qps = {}
                for c in cs:
                    for mt in range(NHL):
                        qps[(c, mt)] = psQ.tile([128, 512], F32,
                                                name=f"qps{c}_{mt}",
                                                tag="q", bufs=8)
                # k-runs into the same PSUM bank (216 ns/matmul chain rate;
                # bank switches cost ~43 ns extra). Half 0 starts k-outer
                # (one matmul per group per k) to track the X DMA stream,
                # then lengthens runs; half 1 has X resident and chains 16.
                kranges = ([(k, k + 1) for k in range(10)]
                           + [(10, 13), (13, 16)]) if half == 0 \
                    else [(0, 16)]

                def runs(c, mt, k0, k1):
                    for k in range(k0, k1):
                        nc.tensor.matmul(
                            qps[(c, mt)],
                            wq[:, k, mt * 128:(mt + 1) * 128],
                            xt[k][:, c * 512:(c + 1) * 512],
                            start=(k == 0), stop=(k == KT - 1))

                def drain(c, mt):
                    sl = slice(c * 512, (c + 1) * 512)
                    # drain PSUM + bias; split across DVE and ACT
                    if mt % 2 == 0:
                        nc.vector.tensor_scalar_add(
                            qth[mt][:, sl], qps[(c, mt)], qb[:, mt:mt + 1])
                    else:
                        nc.scalar.add(qth[mt][:, sl], qps[(c, mt)],
                                      qb[:, mt:mt + 1])

                for k0, k1 in kranges[:-1]:
                    for c in cs:
                        for mt in range(NHL):
                            runs(c, mt, k0, k1)
                # last k-range: finish and drain one c at a time so banks
                # free earlier for the next half / attention PSUM pools
                k0, k1 = kranges[-1]
                for c in cs:
                    for mt in range(NHL):
                        runs(c, mt, k0, k1)
                    for mt in range(NHL):
                        drain(c, mt)
                # rope/transpose for this column half overlaps the next
                # half's matmuls on the PE
                for h in range(NHL):
                    rope_half(h, half)

        # ---- Phase C pool -----------------------------------------------
        with tc.tile_pool(name="pback", bufs=1) as pback:
            ot = [pback.tile([DK, S], F16, name=f"ot{h}") for h in range(NHL)]

            # ---- Phase C: attention (qc-outer) + interleaved Wo ---------
            psS = ctx.enter_context(
                tc.tile_pool(name="psS", bufs=1, space="PSUM"))
            psO = ctx.enter_context(
                tc.tile_pool(name="psO", bufs=1, space="PSUM"))
            psW = ctx.enter_context(
                tc.tile_pool(name="psW", bufs=1, space="PSUM"))

            NC = D // 512
            ats_map = {}

            def emit_scores(h, qc):
                npt = qc * 4 + 4
                ats = []
                # score tiles in [128, 2, 512] pairs (2 PSUM banks) so the
                # exp covers 1024 cols per ACT instruction where possible
                for j in range(npt // 2):
                    sp2 = psS.tile([128, 2, 512], F32, name=f"sp{h}_{qc}_{j}",
                                   tag="sps", bufs=2)
                    at2 = pback.tile([128, 2, 512], BF16,
                                     name=f"at{h}_{qc}_{j}",
                                     tag=f"atp{j}", bufs=3)
                    los = []
                    for jj in range(2):
                        pt = 2 * j + jj
                        off = pt * 128 - qc * 512
                        lo = max(0, off)
                        los.append((lo, off))
                        nc.tensor.matmul(
                            sp2[:, jj, lo:512],
                            qr[h][:, pt * 128:(pt + 1) * 128],
                            qr[h][:, qc * 512 + lo:(qc + 1) * 512],
                            start=True, stop=True)
                    # one exp per pair over [lo_min:512] of both halves; the
                    # extra cols of the later-diagonal half are unwritten
                    # PSUM garbage that is never read downstream
                    lo_min = min(los[0][0], los[1][0])
                    nc.scalar.activation(at2[:, :, lo_min:512],
                                         sp2[:, :, lo_min:512],
                                         AF.Exp, bias=shift_t)
                    for jj in range(2):
                        lo, off = los[jj]
                        if off >= 0:
                            # zero p > q inside the diagonal 128-col block
                            nc.gpsimd.tensor_mul(at2[:, jj, lo:lo + 128],
                                                 at2[:, jj, lo:lo + 128], tri)
                        ats.append((at2, jj, lo, 2 * j + jj))
                ats_map[(h, qc)] = ats

            def emit_dnav(h, qc):
                npt = qc * 4 + 4
                ats = ats_map.pop((h, qc))
                # denominator over p (PE ones-reduce, causal-trimmed);
                # dn rides the sps slot rotation to stay within 8 banks
                dn = psS.tile([1, 512], F32, name=f"dn{h}_{qc}", tag="sps",
                              bufs=2)
                for i, (at2, jj, lo, pt) in enumerate(ats):
                    nc.tensor.matmul(dn[:, lo:512], onecb, at2[:, jj, lo:512],
                                     start=(i == 0), stop=(i == npt - 1))
                lnd = pback.tile([1, 512], F32, name=f"lnd{h}_{qc}",
                                 tag="lnd", bufs=2)
                nc.scalar.activation(lnd, dn, AF.Ln)
                rdr = pback.tile([1, 512], BF16, name=f"rdr{h}_{qc}",
                                 tag="rdr", bufs=2)
                nc.scalar.activation(rdr, lnd, AF.Exp, scale=-1.0)
                bc = psS.tile([128, 512], F32, name=f"bc{h}_{qc}", tag="sps",
                              bufs=2)
                nc.tensor.matmul(bc, onerb, rdr, start=True, stop=True)
                rdb = pback.tile([128, 512], F32, name=f"rdb{h}_{qc}",
                                 tag="rdb", bufs=2)
                nc.vector.tensor_copy(rdb, bc)

                # AV (causal-trimmed) + normalize
                o_ps = psO.tile([128, 512], F32, name=f"ops{h}_{qc}",
                                tag="ops", bufs=2)
                for i, (at2, jj, lo, pt) in enumerate(ats):
                    nc.tensor.matmul(o_ps[:, lo:512], vv[h][:, pt, :],
                                     at2[:, jj, lo:512],
                                     start=(i == 0), stop=(i == npt - 1))
                sl = slice(qc * 512, (qc + 1) * 512)
                nc.vector.tensor_mul(ot[h][:, sl], o_ps, rdb)

            def emit_wo(st):
                osb = pback.tile([128, D], F16, name=f"osb{st}", tag="osb",
                                 bufs=2)
                for ncc in range(NC):
                    wo_ps = psW.tile([128, 512], F32, name=f"wops{st}_{ncc}",
                                     tag="wops", bufs=2)
                    for hh in range(NHL):
                        nc.tensor.matmul(
                            wo_ps,
                            ot[hh][:, st * 128:(st + 1) * 128],
                            wo[:, hh, ncc * 512:(ncc + 1) * 512],
                            start=(hh == 0), stop=(hh == NHL - 1))
                    nsl = slice(ncc * 512, (ncc + 1) * 512)
                    if ncc % 2 == 0:
                        nc.vector.tensor_copy(osb[:, nsl], wo_ps)
                    else:
                        nc.scalar.copy(osb[:, nsl], wo_ps)
                    if ncc % 2 == 1:
                        hsl = slice((ncc - 1) * 512, (ncc + 1) * 512)
                        nc.sync.dma_start(
                            out=dOUT[st * 128:(st + 1) * 128, hsl],
                            in_=osb[:, hsl])

            # pipeline: scores emitted one (h, qc) unit ahead of their
            # denominator/AV so the PE has work during the exp/mask chain
            order = [(h, qc) for qc in range(SK) for h in range(NHL)]
            emit_scores(*order[0])
            emit_scores(*order[1])
            for i, (h, qc) in enumerate(order):
                if i + 2 < len(order):
                    emit_scores(*order[i + 2])
                emit_dnav(h, qc)
                if qc > 0:
                    emit_wo((qc - 1) * 4 + h)
            for st in range((SK - 1) * 4, ST):
                emit_wo(st)

    return nc


# ======================= host-side preparation ===========================

def host_prep(X, Wq_w, Wq_b, Wo_w, Wo_b, rms_w, n_cores=8, NHL=4):
    """Build per-core input maps. X: (B,S,D) fp32."""
    B, S, D = X.shape
    DK = 128
    c = DK ** -0.25
    inv = 1.0 / (ROPE_BASE ** (np.arange(0, DK, 2, dtype=np.float64) / DK))
    ang = np.arange(S, dtype=np.float64)[:, None] * inv[None, :]
    cos = np.concatenate([np.cos(ang), np.cos(ang)], -1)     # (S, DK)
    sin = np.concatenate([np.sin(ang), np.sin(ang)], -1)
    COSA = (cos.T * c).astype(np.float16)                    # (DK, S)
    SINT = (sin.T * c).astype(np.float32)
    SINA = np.concatenate([-SINT[:DK // 2], SINT[DK // 2:]], 0).astype(np.float16)
    TRI = np.triu(np.ones((128, 128), np.float32))                # p <= q
    ONEC = np.ones((128, 1), np.float32)
    ONER = np.ones((1, 128), np.float32)

    # fold rms weight into Wq; fold the rms scale into X (commutes with the
    # projection since it is a per-token scalar)
    Wq_eff = (Wq_w * rms_w[None, :]).astype(np.float32)       # (D, D)
    ms = np.mean(X.astype(np.float32) ** 2, axis=-1, keepdims=True)
    Xn = (X / np.sqrt(ms + EPS)).astype(np.float32)           # (B, S, D)

    in_maps = []
    groups = n_cores // B                                     # head-groups per batch
    ML = NHL * DK
    for core in range(n_cores):
        b = core // groups
        hg = core % groups
        msl = slice(hg * ML, (hg + 1) * ML)
        XT = np.ascontiguousarray(Xn[b].T).astype(np.float16)         # (D, S)
        WQT = np.ascontiguousarray(Wq_eff[msl, :].T).astype(np.float16)   # (D, ML)
        WOT = np.ascontiguousarray(Wo_w[:, msl].T).astype(np.float16)     # (ML, D)
        QB = np.ascontiguousarray(
            Wq_b[msl].reshape(NHL, 128).T).astype(np.float32)             # (128, NHL)
        in_maps.append({
            "XT": XT, "WQT": WQT, "WOT": WOT, "QB": QB,
            "COSA": COSA, "SINA": SINA, "TRI": TRI,
            "ONEC": ONEC, "ONER": ONER,
        })
    return in_maps


def host_reduce(X, Wo_b, results, n_cores=8):
    B, S, D = X.shape
    groups = n_cores // B
    out = np.empty((B, S, D), np.float32)
    for b in range(B):
        acc = X[b].astype(np.float32).copy()
        for hg in range(groups):
            acc += results[b * groups + hg]["OUTP"].astype(np.float32)
        acc += Wo_b[None, :]
        out[b] = acc
    return out


# ======================= public entry point ==============================

_CACHE = {}


def _get_nc():
    if "nc" not in _CACHE:
        nc = build_core(S=2048, D=2048, NHL=4, DK=128, SHIFT=10.0)
        legalize_sync_waits(nc, max_waits=1)
        _CACHE["nc"] = nc
    return _CACHE["nc"]


def kernel(X, Wq_w, Wq_b, Wo_w, Wo_b, rms_w):
    """Full-input MHA block: returns X + MHA(RMSNorm(X)) as np.float32.

    Shards batch(2) x head-groups(4) across 8 NeuronCores; each core
    produces a partial output (its 4 heads through Wo); the host sums the
    four partials per batch and adds bias + residual.
    """
    from concourse.bass_utils import run_bass_kernel_spmd

    X = np.asarray(X, np.float32)
    Wq_w = np.asarray(Wq_w, np.float32)
    Wq_b = np.asarray(Wq_b, np.float32)
    Wo_w = np.asarray(Wo_w, np.float32)
    Wo_b = np.asarray(Wo_b, np.float32)
    rms_w = np.asarray(rms_w, np.float32)

    nc = _get_nc()
    in_maps = host_prep(X, Wq_w, Wq_b, Wo_w, Wo_b, rms_w)
    res = run_bass_kernel_spmd(nc, in_maps, core_ids=list(range(8)))
    return host_reduce(X, Wo_b, res.results)
